# revision 1
# baseline (speedup 1.0000x reference)
# Trainium2 Bass kernel for nn_KokoroModel (text->mel seq2seq, LSTM enc/dec + MHA).
#
# Sharding: data-parallel over batch, 4 examples/core on 8 cores, weights
# replicated. Host-side weight-only fusions (model reparametrizations):
#   TAB  = emb @ enc_Wih.T            (embedding gather -> one-hot matmul)
#   QW   = (Wq @ mproj_in_W) / 8      (mel proj + Q proj + attn scale)
#   M1   = dec_Wih[:, :H] @ mproj_in_W
#   M2   = dec_Wih[:, H:] @ attn_out_W
#   K/V biases absorb tproj_b; decoder gate bias absorbs mproj_in/attn_out biases.
# Softmax normalization is deferred: exp-scores stay unnormalized, row-sums come
# from a ones-column appended to V, context is scaled by 1/sum afterwards
# (valid: scores are tiny, exp cannot overflow).
#
# LSTM steps run weights-stationary on PE: 64 (ldweights+matmul) pairs/step
# accumulate h @ Whh.T into PSUM[128, 16*n] with gates partition-major, then
# ACT sigmoid/tanh and DVE gate algebra. Gate order is torch's [i,f,g,o] in
# 128-row blocks, so sigmoid covers two contiguous spans and tanh one.

import numpy as np
import ml_dtypes

import concourse.bass as bass
import concourse.tile as tile
from concourse import bacc, mybir
from concourse import bass_utils

F32 = mybir.dt.float32
BF16 = mybir.dt.bfloat16
AF = mybir.ActivationFunctionType
BF = ml_dtypes.bfloat16

VOCAB, MEL, H = 256, 80, 512
NH, HD = 8, 64
B_FULL, S_FULL, T_FULL = 32, 512, 1000
NCORES = 8


# ---------------------------------------------------------------------------
# host-side layout helpers
# ---------------------------------------------------------------------------

def _lhsT_tiles(w, kp=128, mp=128):
    """w: [K, M] -> [kp, (K//kp)*(M//mp)*mp]; tile (kc,mc) at cols
    (kc*(M//mp)+mc)*mp."""
    K, M = w.shape
    nk, nm = K // kp, M // mp
    return np.ascontiguousarray(
        w.reshape(nk, kp, nm, mp).transpose(1, 0, 2, 3).reshape(kp, nk * nm * mp))


def _bias_tiles(b, p=128):
    G = b.shape[0]
    return np.ascontiguousarray(b.reshape(G // p, p).T.astype(np.float32))


# ---------------------------------------------------------------------------
# device program
# ---------------------------------------------------------------------------

def build_program(S=S_FULL, T=T_FULL, n=B_FULL // NCORES, stages=5):
    nc = bacc.Bacc("TRN2", target_bir_lowering=False, debug=False)

    NS, NT = n * S, n * T
    SN = S            # text matmul N-tile (== S so each tile is one example)
    TN = T // 2       # mel matmul N-tile (1000 -> 500)
    assert SN <= 512 and TN <= 512 and TN * 4 <= 2048

    d = {}

    def din(name, shape, dt):
        d[name] = nc.dram_tensor(name, list(shape), dt, kind="ExternalInput")

    din("oh_f", (128, 2 * NS), BF16)
    din("oh_b", (128, 2 * NS), BF16)
    din("tab_f", (128, 2 * 2048), BF16)
    din("tab_b", (128, 2 * 2048), BF16)
    din("whh_f", (128, 4 * 16 * 128), BF16)
    din("whh_b", (128, 4 * 16 * 128), BF16)
    din("whh_d", (128, 4 * 16 * 128), BF16)
    din("wt_f", (128, 4 * 4 * 128), BF16)
    din("wt_b", (128, 4 * 4 * 128), BF16)
    din("tb", (128, 4), F32)
    din("wk", (128, 4 * 4 * 128), BF16)
    din("wv", (128, 4 * 512), BF16)
    din("vb", (64, 8), F32)
    din("qw", (81, 4 * 128), BF16)
    din("melT", (81, NT), BF16)
    din("m1", (81, 2048), BF16)
    din("m2", (128, 4 * 2048), BF16)
    din("mo", (128, 4 * 80), BF16)
    din("mob", (80, 1), F32)

    out = nc.dram_tensor("out", [80, NT], F32, kind="ExternalOutput")

    xg_f = nc.dram_tensor("xg_f", [NS, 2048], BF16, kind="Internal")
    xg_b = nc.dram_tensor("xg_b", [NS, 2048], BF16, kind="Internal")
    xg_d = nc.dram_tensor("xg_d", [NT, 2048], BF16, kind="Internal")

    with tile.TileContext(nc) as tc:
        _body(tc, nc, d, out, xg_f, xg_b, xg_d, S, T, n, NS, NT, SN, TN, stages)

    nc.compile()
    return nc


def _bias_copy(nc, i, dst, src, bias_col):
    """psum->sbuf cast + per-partition bias; alternate ACT/DVE for throughput."""
    if i % 2 == 0:
        nc.scalar.activation(dst, src, AF.Identity, bias=bias_col)
    else:
        nc.vector.tensor_scalar_add(dst, src, bias_col)


def _plain_copy(nc, i, dst, src):
    if i % 2 == 0:
        nc.scalar.copy(dst, src)
    else:
        nc.vector.tensor_copy(dst, src)


def _load(nc, pool, d, name, shape, dt):
    t_ = pool.tile(list(shape), dt, tag=name)
    nc.sync.dma_start(t_[:, :], d[name].ap())
    return t_


def _xg_produce(tc, nc, psp, sbp, tab, oh, xg_dram, S, n, NS, SN):
    """xg[b*S+t, g] = (onehot.T @ TAB')[token, g]; bias pre-folded into TAB'.
    Token-major layout: contiguous DMA runs both directions."""
    i = 0
    for b in range(n):
        for tcx in range(S // 128):
            row0 = b * S + tcx * 128
            for gs in range(4):
                ps = psp.tile([128, 512], F32, tag="ps")
                for vc in range(2):
                    nc.tensor.matmul(
                        ps[:, :],
                        oh[:, vc * NS + row0:vc * NS + row0 + 128],
                        tab[:, vc * 2048 + gs * 512:vc * 2048 + (gs + 1) * 512],
                        start=(vc == 0), stop=(vc == 1))
                sb = sbp.tile([128, 512], BF16, tag="xg_sb")
                _plain_copy(nc, i, sb[:, :], ps[:, :])
                i += 1
                nc.sync.dma_start(
                    xg_dram.ap()[row0:row0 + 128, gs * 512:(gs + 1) * 512], sb[:, :])


def _lstm(pools, nc, T_steps, n, xg_dram, whh, outbuf, T_buf, col_of_t, PF=8):
    """LSTM recurrence; h_t (bf16) lands in outbuf[:, kc*(T_buf*n) + b*T_buf +
    col_of_t(t)].  xg_dram: [T,16,128,n], gate blocks [i,f,g,o] x 4 chunks."""
    psum_p, state_p, gate_p, h_p, tmp_p, xg_p = pools
    xg_ap = xg_dram.ap().rearrange("(b t) (gc p) -> p t gc b", b=n, gc=16)
    ob = outbuf[:, :].rearrange("p (kc b t) -> p kc b t", kc=4, b=n)

    c = state_p.tile([128, 4 * n], F32, tag="c_state")
    nc.vector.memset(c[:, :], 0.0)
    h = state_p.tile([128, 4 * n], BF16, tag="h_init")
    nc.vector.memset(h[:, :], 0.0)

    gw = 4 * n
    xgs = None
    for t in range(T_steps):
        if t % PF == 0:
            pf = min(PF, T_steps - t)
            xgs = xg_p.tile([128, PF * 16 * n], BF16, tag="xg_stream")
            xv = xgs[:, :].rearrange("p (t g b) -> p t g b", t=PF, g=16)
            for b in range(n):
                nc.sync.dma_start(xv[:, :pf, :, b], xg_ap[:, t:t + pf, :, b])
        toff = (t % PF) * 16 * n

        ps = psum_p.tile([128, 16 * n], F32, tag="ps")
        for gc in range(16):
            for kc in range(4):
                nc.tensor.matmul(
                    ps[:, gc * n:(gc + 1) * n],
                    whh[:, (kc * 16 + gc) * 128:(kc * 16 + gc + 1) * 128],
                    h[:, kc * n:(kc + 1) * n],
                    start=(kc == 0), stop=(kc == 3))

        gates = gate_p.tile([128, 16 * n], F32, tag="gates")
        nc.vector.tensor_add(gates[:, :], ps[:, :], xgs[:, toff:toff + 16 * n])
        acts = gate_p.tile([128, 16 * n], F32, tag="acts")
        nc.scalar.activation(acts[:, 0:2 * gw], gates[:, 0:2 * gw], AF.Sigmoid)
        nc.scalar.activation(acts[:, 2 * gw:3 * gw], gates[:, 2 * gw:3 * gw], AF.Tanh)
        nc.scalar.activation(acts[:, 3 * gw:4 * gw], gates[:, 3 * gw:4 * gw], AF.Sigmoid)

        t1 = tmp_p.tile([128, gw], F32, tag="t1")
        nc.vector.tensor_mul(t1[:, :], acts[:, gw:2 * gw], c[:, :])
        t2 = tmp_p.tile([128, gw], F32, tag="t2")
        nc.vector.tensor_mul(t2[:, :], acts[:, 0:gw], acts[:, 2 * gw:3 * gw])
        nc.vector.tensor_add(c[:, :], t1[:, :], t2[:, :])
        tnh = tmp_p.tile([128, gw], F32, tag="tanh_c")
        nc.scalar.activation(tnh[:, :], c[:, :], AF.Tanh)
        h_new = h_p.tile([128, gw], BF16, tag="h_new")
        nc.vector.tensor_mul(h_new[:, :], acts[:, 3 * gw:4 * gw], tnh[:, :])

        nc.sync.dma_start(ob[:, :, :, col_of_t(t)],
                          h_new[:, :].rearrange("p (kc b) -> p kc b", kc=4))
        h = h_new


def _body(tc, nc, d, out, xg_f, xg_b, xg_d, S, T, n, NS, NT, SN, TN, stages=5):
    n_sn = NS // SN
    n_tn = NT // TN

    with tc.tile_pool(name="persist", bufs=1) as pp:
        melT = _load(nc, pp, d, "melT", (81, NT), BF16)
        te = pp.tile([128, 4 * NS], BF16, tag="te")
        ctx = pp.tile([128, 4 * NT], BF16, tag="ctx")

        # ---------------- phase E0: encoder gate precompute ----------------
        with tc.tile_pool(name="e0w", bufs=1) as wp, \
             tc.tile_pool(name="e0ps", bufs=4, space="PSUM") as psp, \
             tc.tile_pool(name="e0sb", bufs=4) as sbp:
            tab_f = _load(nc, wp, d, "tab_f", (128, 4096), BF16)
            tab_b = _load(nc, wp, d, "tab_b", (128, 4096), BF16)
            oh_f = _load(nc, wp, d, "oh_f", (128, 2 * NS), BF16)
            oh_b = _load(nc, wp, d, "oh_b", (128, 2 * NS), BF16)
            _xg_produce(tc, nc, psp, sbp, tab_f, oh_f, xg_f, S, n, NS, SN)
            _xg_produce(tc, nc, psp, sbp, tab_b, oh_b, xg_b, S, n, NS, SN)
        if stages <= 1:
            return

        # ---------------- phase E1+E2: encoder recurrences + tproj ---------
        with tc.tile_pool(name="e1w", bufs=1) as ewp, \
             tc.tile_pool(name="e1buf", bufs=1) as ebp, \
             tc.tile_pool(name="e1ps", bufs=4, space="PSUM") as pls, \
             tc.tile_pool(name="e1st", bufs=1) as stp, \
             tc.tile_pool(name="e1gt", bufs=3) as gtp, \
             tc.tile_pool(name="e1h", bufs=3) as hp, \
             tc.tile_pool(name="e1tmp", bufs=3) as tmp, \
             tc.tile_pool(name="e1xg", bufs=3) as xgp:

            whh_f = _load(nc, ewp, d, "whh_f", (128, 8192), BF16)
            whh_b = _load(nc, ewp, d, "whh_b", (128, 8192), BF16)
            buf_f = ebp.tile([128, 4 * NS], BF16, tag="buf_f")
            buf_b = ebp.tile([128, 4 * NS], BF16, tag="buf_b")

            lp = (pls, stp, gtp, hp, tmp, xgp)
            _lstm(lp, nc, S, n, xg_f, whh_f, buf_f, S, lambda t: t)
            _lstm(lp, nc, S, n, xg_b, whh_b, buf_b, S, lambda t: S - 1 - t)

            with tc.tile_pool(name="e2w", bufs=1) as wtp, \
                 tc.tile_pool(name="e2ps", bufs=4, space="PSUM") as ptp:
                wt_f = _load(nc, wtp, d, "wt_f", (128, 2048), BF16)
                wt_b = _load(nc, wtp, d, "wt_b", (128, 2048), BF16)
                tb = _load(nc, wtp, d, "tb", (128, 4), F32)
                for mc in range(4):
                    for nt in range(n_sn):
                        ps = ptp.tile([128, SN], F32, tag="ps")
                        for kc in range(4):
                            nc.tensor.matmul(
                                ps[:, :],
                                wt_f[:, (kc * 4 + mc) * 128:(kc * 4 + mc + 1) * 128],
                                buf_f[:, kc * NS + nt * SN:kc * NS + (nt + 1) * SN],
                                start=(kc == 0), stop=False)
                        for kc in range(4):
                            nc.tensor.matmul(
                                ps[:, :],
                                wt_b[:, (kc * 4 + mc) * 128:(kc * 4 + mc + 1) * 128],
                                buf_b[:, kc * NS + nt * SN:kc * NS + (nt + 1) * SN],
                                start=False, stop=(kc == 3))
                        _bias_copy(nc, mc * n_sn + nt,
                                   te[:, mc * NS + nt * SN:mc * NS + (nt + 1) * SN],
                                   ps[:, :], tb[:, mc:mc + 1])

        if stages <= 2:
            return
        # ---------------- phase A: attention ------------------------------
        _attention(tc, nc, d, te, ctx, melT, S, T, n, NS, NT, SN, TN)

        if stages <= 3:
            return
        # ---------------- phase X: decoder gate precompute ----------------
        with tc.tile_pool(name="xw", bufs=1) as dwp, \
             tc.tile_pool(name="xps", bufs=4, space="PSUM") as pdx, \
             tc.tile_pool(name="xsb", bufs=4) as dsb:
            m1 = _load(nc, dwp, d, "m1", (81, 2048), BF16)
            m2 = _load(nc, dwp, d, "m2", (128, 4 * 2048), BF16)
            tchunks = [(t0, min(128, T - t0)) for t0 in range(0, T, 128)]
            i = 0
            for b in range(n):
                for (t0, tsz) in tchunks:
                    row0 = b * T + t0
                    for gs in range(4):
                        ps = pdx.tile([128, 512], F32, tag="ps")
                        nc.tensor.matmul(
                            ps[:tsz, :], melT[:, row0:row0 + tsz],
                            m1[:, gs * 512:(gs + 1) * 512],
                            start=True, stop=False)
                        for kc in range(4):
                            nc.tensor.matmul(
                                ps[:tsz, :],
                                ctx[:, kc * NT + row0:kc * NT + row0 + tsz],
                                m2[:, kc * 2048 + gs * 512:kc * 2048 + (gs + 1) * 512],
                                start=False, stop=(kc == 3))
                        sb = dsb.tile([128, 512], BF16, tag="dx_sb")
                        _plain_copy(nc, i, sb[:tsz, :], ps[:tsz, :])
                        i += 1
                        nc.sync.dma_start(
                            xg_d.ap()[row0:row0 + tsz, gs * 512:(gs + 1) * 512],
                            sb[:tsz, :])

        if stages <= 4:
            return
        # ---------------- phase D: decoder recurrence + out proj ----------
        with tc.tile_pool(name="dw", bufs=1) as dwp, \
             tc.tile_pool(name="dbuf", bufs=1) as dbp, \
             tc.tile_pool(name="dps", bufs=4, space="PSUM") as pls, \
             tc.tile_pool(name="dst", bufs=1) as stp, \
             tc.tile_pool(name="dgt", bufs=3) as gtp, \
             tc.tile_pool(name="dh", bufs=3) as hp, \
             tc.tile_pool(name="dtmp", bufs=3) as tmp, \
             tc.tile_pool(name="dxg", bufs=3) as xgp:

            whh_d = _load(nc, dwp, d, "whh_d", (128, 8192), BF16)
            dbuf = dbp.tile([128, 4 * NT], BF16, tag="dbuf")
            lp = (pls, stp, gtp, hp, tmp, xgp)
            _lstm(lp, nc, T, n, xg_d, whh_d, dbuf, T, lambda t: t)

            with tc.tile_pool(name="ow", bufs=1) as mop, \
                 tc.tile_pool(name="ops", bufs=4, space="PSUM") as pso, \
                 tc.tile_pool(name="osb", bufs=4) as sbo:
                mo = _load(nc, mop, d, "mo", (128, 320), BF16)
                mob = _load(nc, mop, d, "mob", (80, 1), F32)
                for nt in range(n_tn):
                    ps = pso.tile([80, TN], F32, tag="ps")
                    for kc in range(4):
                        nc.tensor.matmul(
                            ps[:, :], mo[:, kc * 80:(kc + 1) * 80],
                            dbuf[:, kc * NT + nt * TN:kc * NT + (nt + 1) * TN],
                            start=(kc == 0), stop=(kc == 3))
                    sb = sbo.tile([80, TN], F32, tag="out_sb")
                    nc.scalar.activation(sb[:, :], ps[:, :], AF.Identity,
                                         bias=mob[:, :])
                    nc.sync.dma_start(out.ap()[:, nt * TN:(nt + 1) * TN], sb[:, :])


def _attention(tc, nc, d, te, ctx, melT, S, T, n, NS, NT, SN, TN):
    n_tn = NT // TN
    n_sc = NS // 128
    with tc.tile_pool(name="aw", bufs=1) as awp, \
         tc.tile_pool(name="aps", bufs=4, space="PSUM") as pa, \
         tc.tile_pool(name="asb", bufs=4) as asb, \
         tc.tile_pool(name="aqt", bufs=1) as qtp, \
         tc.tile_pool(name="akt", bufs=1) as ktp, \
         tc.tile_pool(name="avs", bufs=1) as vsp, \
         tc.tile_pool(name="aet", bufs=2) as etp, \
         tc.tile_pool(name="actx", bufs=3) as cxp:

        wk = _load(nc, awp, d, "wk", (128, 2048), BF16)
        wv = _load(nc, awp, d, "wv", (128, 2048), BF16)
        vb = _load(nc, awp, d, "vb", (64, 8), F32)
        qw = _load(nc, awp, d, "qw", (81, 512), BF16)
        ones = awp.tile([1, 64], F32, tag="ones64")
        nc.vector.memset(ones[:, :], 1.0)

        # QT sbuf-resident: [128, 4mc x NT]
        qt = qtp.tile([128, 4 * NT], BF16, tag="qt")
        for mc in range(4):
            for nt in range(n_tn):
                ps = pa.tile([128, TN], F32, tag="ps")
                nc.tensor.matmul(ps[:, :], qw[:, mc * 128:(mc + 1) * 128],
                                 melT[:, nt * TN:(nt + 1) * TN],
                                 start=True, stop=True)
                _plain_copy(nc, mc * n_tn + nt,
                            qt[:, mc * NT + nt * TN:mc * NT + (nt + 1) * TN],
                            ps[:, :])

        # KT sbuf-resident: [128, 4mc x NS]
        kt = ktp.tile([128, 4 * NS], BF16, tag="kt")
        for mc in range(4):
            for nt in range(NS // SN):
                ps = pa.tile([128, SN], F32, tag="ps")
                for kc in range(4):
                    nc.tensor.matmul(
                        ps[:, :], wk[:, (kc * 4 + mc) * 128:(kc * 4 + mc + 1) * 128],
                        te[:, kc * NS + nt * SN:kc * NS + (nt + 1) * SN],
                        start=(kc == 0), stop=(kc == 3))
                _plain_copy(nc, mc * (NS // SN) + nt,
                            kt[:, mc * NS + nt * SN:mc * NS + (nt + 1) * SN],
                            ps[:, :])

        # V with ones column per head: [128(s-sub), n_sc x (8h x 65)]
        vsb = vsp.tile([128, n_sc * 520], BF16, tag="vsb")
        for sc in range(n_sc):
            ps = pa.tile([128, 512], F32, tag="ps")
            for kc in range(4):
                nc.tensor.matmul(
                    ps[:, :], te[:, kc * NS + sc * 128:kc * NS + sc * 128 + 128],
                    wv[:, kc * 512:(kc + 1) * 512],
                    start=(kc == 0), stop=(kc == 3))
            dst = vsb[:, sc * 520:(sc + 1) * 520].rearrange("p (h c) -> p h c", h=8)
            _plain_copy(nc, sc, dst[:, :, 0:64],
                        ps[:, :].rearrange("p (h c) -> p h c", h=8))
            nc.vector.memset(dst[:, :, 64:65], 1.0)

        # per (example, head): scoresT -> exp -> ctx + sums -> scale
        nsc_b = S // 128  # s-chunks per example
        for b in range(n):
            for h in range(NH):
                hc, hr = h // 2, (h % 2) * 64
                et = etp.tile([128, nsc_b * T], BF16, tag="et")
                for tt in range(T // TN):
                    qs = qt[hr:hr + 64,
                            hc * NT + b * T + tt * TN:hc * NT + b * T + (tt + 1) * TN]
                    for scl in range(nsc_b):
                        ps = pa.tile([128, TN], F32, tag="ps")
                        nc.tensor.matmul(
                            ps[:, :],
                            kt[hr:hr + 64,
                               hc * NS + b * S + scl * 128:hc * NS + b * S + scl * 128 + 128],
                            qs, start=True, stop=True)
                        nc.scalar.activation(
                            et[:, scl * T + tt * TN:scl * T + (tt + 1) * TN],
                            ps[:, :], AF.Exp)
                cps = [pa.tile([65, TN], F32, tag="ps", name=f"cps{tt}")
                       for tt in range(T // TN)]
                for scl in range(nsc_b):
                    lhs = vsb[:, (b * nsc_b + scl) * 520 + h * 65:
                              (b * nsc_b + scl) * 520 + (h + 1) * 65]
                    for tt in range(T // TN):
                        nc.tensor.matmul(cps[tt][:, :], lhs,
                                         et[:, scl * T + tt * TN:scl * T + (tt + 1) * TN],
                                         start=(scl == 0), stop=(scl == nsc_b - 1))
                for tt in range(T // TN):
                    rc = cxp.tile([1, TN], F32, tag="recip")
                    nc.vector.reciprocal(rc[:, :], cps[tt][64:65, :])
                    rb = pa.tile([64, TN], F32, tag="ps")
                    nc.tensor.matmul(rb[:, :], ones[:, :], rc[:, :],
                                     start=True, stop=True)
                    c0 = cxp.tile([64, TN], F32, tag="ctx_unsc")
                    _plain_copy(nc, b * NH + h + tt, c0[:, :], cps[tt][0:64, :])
                    sc1 = cxp.tile([64, TN], F32, tag="ctx_scaled")
                    nc.vector.tensor_mul(sc1[:, :], c0[:, :], rb[:, :])
                    nc.vector.tensor_scalar_add(
                        ctx[hr:hr + 64,
                            hc * NT + b * T + tt * TN:hc * NT + b * T + (tt + 1) * TN],
                        sc1[:, :], vb[:, h:h + 1])


# ---------------------------------------------------------------------------
# host wrapper
# ---------------------------------------------------------------------------

def prep_host(inputs, S, T, n_per_core, ncores):
    f32 = np.float32
    emb = np.asarray(inputs["emb"], f32)
    idx = np.asarray(inputs["phoneme_indices"]).astype(np.int64)
    mel = np.asarray(inputs["mel_specs"], f32)

    bias_f = np.asarray(inputs["enc_bih_f"], f32) + np.asarray(inputs["enc_bhh_f"], f32)
    bias_b = np.asarray(inputs["enc_bih_b"], f32) + np.asarray(inputs["enc_bhh_b"], f32)
    tab_f = emb @ np.asarray(inputs["enc_Wih_f"], f32).T + bias_f
    tab_b = emb @ np.asarray(inputs["enc_Wih_b"], f32).T + bias_b

    tproj_W = np.asarray(inputs["tproj_W"], f32)
    tproj_b = np.asarray(inputs["tproj_b"], f32)
    Wq, Wk, Wv = np.split(np.asarray(inputs["attn_in_W"], f32), 3, axis=0)
    bq, bk, bv = np.split(np.asarray(inputs["attn_in_b"], f32), 3)
    mpw = np.asarray(inputs["mproj_in_W"], f32)
    mpb = np.asarray(inputs["mproj_in_b"], f32)
    aow = np.asarray(inputs["attn_out_W"], f32)
    aob = np.asarray(inputs["attn_out_b"], f32)
    dWih = np.asarray(inputs["dec_Wih"], f32)
    dbias = np.asarray(inputs["dec_bih"], f32) + np.asarray(inputs["dec_bhh"], f32)
    mow = np.asarray(inputs["mproj_out_W"], f32)
    mob = np.asarray(inputs["mproj_out_b"], f32)

    scale = f32(1.0) / np.sqrt(f32(HD))
    QW = (Wq @ mpw) * scale
    qb_ = (bq + Wq @ mpb) * scale
    vb_ = bv                       # tproj_b folded into te; bk softmax-invariant
    W1, W2 = dWih[:, :H], dWih[:, H:]
    M1 = W1 @ mpw
    M2 = W2 @ aow
    dbias_ = dbias + W1 @ mpb + W2 @ aob

    def bf(a):
        return np.ascontiguousarray(a.astype(BF))

    common = {
        "tab_f": bf(np.concatenate([tab_f[:128], tab_f[128:]], axis=1)),
        "tab_b": bf(np.concatenate([tab_b[:128], tab_b[128:]], axis=1)),
        "whh_f": bf(_lhsT_tiles(np.asarray(inputs["enc_Whh_f"], f32).T)),
        "whh_b": bf(_lhsT_tiles(np.asarray(inputs["enc_Whh_b"], f32).T)),
        "whh_d": bf(_lhsT_tiles(np.asarray(inputs["dec_Whh"], f32).T)),
        "wt_f": bf(_lhsT_tiles(tproj_W[:, :H].T)),
        "wt_b": bf(_lhsT_tiles(tproj_W[:, H:].T)),
        "tb": _bias_tiles(tproj_b),
        "wk": bf(_lhsT_tiles(Wk.T)),
        "wv": bf(Wv.T.reshape(4, 128, 512).transpose(1, 0, 2).reshape(128, 2048)),
        "vb": np.ascontiguousarray(vb_.reshape(8, 64).T.astype(f32)),
        "qw": bf(np.concatenate([QW.T, qb_.reshape(1, 512)], axis=0)),
        "m1": bf(np.concatenate([M1.T, dbias_.reshape(1, 2048)], axis=0)),
        "m2": bf(M2.T.reshape(4, 128, 2048).transpose(1, 0, 2).reshape(128, 4 * 2048)),
        "mo": bf(mow.T.reshape(4, 128, 80).transpose(1, 0, 2).reshape(128, 320)),
        "mob": np.ascontiguousarray(mob.reshape(80, 1).astype(f32)),
    }

    shifted = np.concatenate([np.zeros_like(mel[:, :1]), mel[:, :-1]], axis=1)

    in_maps = []
    for c in range(ncores):
        exs = list(range(c * n_per_core, (c + 1) * n_per_core))
        ohf = np.zeros((VOCAB, n_per_core * S), f32)
        ohb = np.zeros((VOCAB, n_per_core * S), f32)
        cols = np.arange(S)
        for bi, e in enumerate(exs):
            ohf[idx[e, :S], bi * S + cols] = 1.0
            ohb[idx[e, S - 1 - cols], bi * S + cols] = 1.0
        melTc = np.ones((MEL + 1, n_per_core * T), f32)
        for bi, e in enumerate(exs):
            melTc[:MEL, bi * T:(bi + 1) * T] = shifted[e, :T].T
        m = dict(common)
        m["oh_f"] = bf(np.concatenate([ohf[:128], ohf[128:]], axis=1))
        m["oh_b"] = bf(np.concatenate([ohb[:128], ohb[128:]], axis=1))
        m["melT"] = bf(melTc)
        in_maps.append(m)
    return in_maps


def run(inputs, S, T, n, ncores, trace=False):
    nc = build_program(S=S, T=T, n=n)
    in_maps = prep_host(inputs, S, T, n, ncores)
    res = bass_utils.run_bass_kernel_spmd(
        nc, in_maps, core_ids=list(range(ncores)), trace=trace)
    Bt = n * ncores
    out = np.zeros((Bt, T, MEL), np.float32)
    for c in range(ncores):
        o = np.asarray(res.results[c]["out"])
        for bi in range(n):
            out[c * n + bi] = o[:, bi * T:(bi + 1) * T].T
    return out, res


def kernel(**inputs):
    out, _ = run(inputs, S_FULL, T_FULL, B_FULL // NCORES, NCORES)
    return out



# revision 4
# speedup vs baseline: 1.0873x; 1.0873x over previous
# Trainium2 Bass kernel for nn_KokoroModel (text->mel seq2seq, LSTM enc/dec + MHA).
#
# Sharding: data-parallel over batch, 4 examples/core on 8 cores, weights
# replicated. Host-side weight-only fusions (model reparametrizations):
#   TAB  = emb @ enc_Wih.T            (embedding gather -> one-hot matmul)
#   QW   = (Wq @ mproj_in_W) / 8      (mel proj + Q proj + attn scale)
#   M1   = dec_Wih[:, :H] @ mproj_in_W
#   M2   = dec_Wih[:, H:] @ attn_out_W
#   K/V biases absorb tproj_b; decoder gate bias absorbs mproj_in/attn_out biases.
# Softmax normalization is deferred: exp-scores stay unnormalized, row-sums come
# from a ones-column appended to V, context is scaled by 1/sum afterwards
# (valid: scores are tiny, exp cannot overflow).
#
# LSTM step engine schedule (gates host-reordered to [i,f,o,g]):
#   PE:  16 identity matmuls inject xg_t into PSUM (no h dependency, can run
#        during the previous step's vector phase), then 64 (ldweights,matmul)
#        pairs accumulate h @ Whh.T on top.
#   ACT: sigmoid over [i,f,o] (one op), tanh(g) into state's tg slot.
#   DVE: one mul [i'|f'] * [tg|c], one add -> c, ACT tanh(c), one mul o'*tanh(c)
#        writing h (bf16) straight into the sequence output buffer (no DMA).
# Encoder fwd/bwd chains are interleaved in a single 512-iteration loop so the
# two dependency chains fill each other's engine gaps. Gate pre-activations
# (xg) are produced gate-major in DRAM by transposed-orientation matmuls
# (E0 / phase X) and streamed into SBUF in large double-buffered chunks.

import numpy as np
import ml_dtypes

import concourse.bass as bass
import concourse.tile as tile
from concourse import bacc, mybir
from concourse import bass_utils

F32 = mybir.dt.float32
BF16 = mybir.dt.bfloat16
AF = mybir.ActivationFunctionType
BF = ml_dtypes.bfloat16

VOCAB, MEL, H = 256, 80, 512
NH, HD = 8, 64
B_FULL, S_FULL, T_FULL = 32, 512, 1000
NCORES = 8

INJECT = True  # xg into PSUM via identity matmul (False: DVE add)


# ---------------------------------------------------------------------------
# host-side layout helpers
# ---------------------------------------------------------------------------

def _lhsT_tiles(w, kp=128, mp=128):
    """w: [K, M] -> [kp, (K//kp)*(M//mp)*mp]; tile (kc,mc) at cols
    (kc*(M//mp)+mc)*mp."""
    K, M = w.shape
    nk, nm = K // kp, M // mp
    return np.ascontiguousarray(
        w.reshape(nk, kp, nm, mp).transpose(1, 0, 2, 3).reshape(kp, nk * nm * mp))


def _bias_tiles(b, p=128):
    G = b.shape[0]
    return np.ascontiguousarray(b.reshape(G // p, p).T.astype(np.float32))


def _gate_cols(a):
    """reorder last-axis gate blocks [i,f,g,o] -> [i,f,o,g]."""
    G = a.shape[-1] // 4
    return np.concatenate(
        [a[..., 0:G], a[..., G:2 * G], a[..., 3 * G:4 * G], a[..., 2 * G:3 * G]],
        axis=-1)


# ---------------------------------------------------------------------------
# device program
# ---------------------------------------------------------------------------

def build_program(S=S_FULL, T=T_FULL, n=B_FULL // NCORES, stages=5):
    nc = bacc.Bacc("TRN2", target_bir_lowering=False, debug=False)

    NS, NT = n * S, n * T
    SN = S            # text matmul N-tile (== S so each tile is one example)
    TN = T // 2       # mel matmul N-tile (1000 -> 500)
    assert SN <= 512 and TN <= 512

    d = {}

    def din(name, shape, dt):
        d[name] = nc.dram_tensor(name, list(shape), dt, kind="ExternalInput")

    din("oh_f", (128, 2 * NS), BF16)
    din("oh_b", (128, 2 * NS), BF16)
    din("tab_f", (128, 2 * 16 * 128), BF16)   # lhsT tiles (vc, gt)
    din("tab_b", (128, 2 * 16 * 128), BF16)
    din("whh_f", (128, 4 * 16 * 128), BF16)
    din("whh_b", (128, 4 * 16 * 128), BF16)
    din("whh_d", (128, 4 * 16 * 128), BF16)
    din("wt_f", (128, 4 * 4 * 128), BF16)
    din("wt_b", (128, 4 * 4 * 128), BF16)
    din("tb", (128, 4), F32)
    din("wk", (128, 4 * 4 * 128), BF16)
    din("wv", (128, 4 * 512), BF16)
    din("vb", (64, 8), F32)
    din("qw", (81, 4 * 128), BF16)
    din("melT", (81, NT), BF16)
    din("m1", (81, 16 * 128), BF16)           # lhsT tiles (gt)
    din("m2", (128, 4 * 16 * 128), BF16)      # lhsT tiles (kc, gt)
    din("mo", (128, 4 * 80), BF16)
    din("mob", (80, 1), F32)
    din("ident", (128, 128), BF16)

    out = nc.dram_tensor("out", [80, NT], F32, kind="ExternalOutput")

    # gate-major xg: col = gc*(n*S) + b*S + t
    xg_f = nc.dram_tensor("xg_f", [128, 16 * NS], BF16, kind="Internal")
    xg_b = nc.dram_tensor("xg_b", [128, 16 * NS], BF16, kind="Internal")
    xg_d = nc.dram_tensor("xg_d", [128, 16 * NT], BF16, kind="Internal")

    with tile.TileContext(nc) as tc:
        _body(tc, nc, d, out, xg_f, xg_b, xg_d, S, T, n, NS, NT, SN, TN, stages)

    nc.compile()
    return nc


def _bias_copy(nc, i, dst, src, bias_col):
    if i % 2 == 0:
        nc.scalar.activation(dst, src, AF.Identity, bias=bias_col)
    else:
        nc.vector.tensor_scalar_add(dst, src, bias_col)


def _plain_copy(nc, i, dst, src):
    if i % 2 == 0:
        nc.scalar.copy(dst, src)
    else:
        nc.vector.tensor_copy(dst, src)


def _load(nc, pool, d, name, shape, dt):
    t_ = pool.tile(list(shape), dt, tag=name)
    nc.sync.dma_start(t_[:, :], d[name].ap())
    return t_


def _xg_produce(tc, nc, psp, sbp, tab, oh, xg_dram, S, n, NS):
    """xg[128g, (gt,b,t)] = sum_vc tab_tile(vc,gt).T @ oh[vc, b-span].
    Gate-major output so the LSTM streams contiguous chunks."""
    xgv = xg_dram.ap().rearrange("p (gc b t) -> p gc b t", gc=16, b=n)
    i = 0
    for gt in range(16):
        for b in range(n):
            ps = psp.tile([128, 512], F32, tag="ps")
            for vc in range(2):
                nc.tensor.matmul(
                    ps[:, :S],
                    tab[:, (vc * 16 + gt) * 128:(vc * 16 + gt + 1) * 128],
                    oh[:, vc * NS + b * S:vc * NS + (b + 1) * S],
                    start=(vc == 0), stop=(vc == 1))
            sb = sbp.tile([128, 512], BF16, tag="xg_sb")
            _plain_copy(nc, i, sb[:, :S], ps[:, :S])
            i += 1
            nc.sync.dma_start(xgv[:, gt, b, :], sb[:, :S])


class _Chain:
    """One LSTM recurrence's persistent tiles + per-step tags."""

    def __init__(self, name, whh, xg_dram, outbuf, n, S, ident,
                 state_p, xg_pool, CH):
        self.name = name
        self.whh = whh
        self.xgv = xg_dram.ap().rearrange("p (gc b t) -> p gc b t", gc=16, b=n)
        self.ob = outbuf[:, :].rearrange("p (kc b t) -> p kc b t", kc=4, b=n)
        self.n = n
        self.S = S
        self.ident = ident
        self.CH = CH
        self.gw = 4 * n
        self.state = state_p.tile([128, 2 * self.gw], F32, tag=f"st_{name}")
        self.chunks = [None, None]
        self.xg_pool = xg_pool

    def init(self, nc):
        nc.vector.memset(self.state[:, self.gw:2 * self.gw], 0.0)  # c slot
        self.load_chunk(nc, 0)

    def load_chunk(self, nc, ci):
        ch = min(self.CH, self.S - ci * self.CH)
        t_ = self.xg_pool.tile([128, 16 * self.n * self.CH], BF16,
                               tag=f"xgc_{self.name}")
        v = t_[:, :].rearrange("p (gc b t) -> p gc b t", gc=16, b=self.n)
        nc.sync.dma_start(v[:, :, :, :ch],
                          self.xgv[:, :, :, ci * self.CH:ci * self.CH + ch])
        self.chunks[ci % 2] = v

    def step(self, nc, t, psum_p, gate_p, col_of_t):
        n, gw, CH = self.n, self.gw, self.CH
        if t % CH == 0 and t + CH < self.S:
            self.load_chunk(nc, t // CH + 1)
        xgc = self.chunks[(t // CH) % 2]
        tl = t % CH

        ps = psum_p.tile([128, 4 * gw], F32, tag=f"ps_{self.name}")
        xg_t = xgc[:, :, :, tl].rearrange("p gc b -> p (gc b)")
        if INJECT:
            nc.tensor.matmul(ps[:, :], self.ident[:, :], xg_t,
                             start=True, stop=(t == 0))
        if t > 0:
            hv = self.ob[:, :, :, col_of_t(t - 1)]
            if INJECT:
                for kc in range(4):
                    for gc in range(16):
                        nc.tensor.matmul(
                            ps[:, gc * n:(gc + 1) * n],
                            self.whh[:, (kc * 16 + gc) * 128:(kc * 16 + gc + 1) * 128],
                            hv[:, kc, :],
                            start=False, stop=(kc == 3 and gc == 15))
            else:
                for gc in range(16):
                    for kc in range(4):
                        nc.tensor.matmul(
                            ps[:, gc * n:(gc + 1) * n],
                            self.whh[:, (kc * 16 + gc) * 128:(kc * 16 + gc + 1) * 128],
                            hv[:, kc, :],
                            start=(kc == 0), stop=(kc == 3))

        if INJECT:
            gsrc = ps
        else:
            gates = gate_p.tile([128, 4 * gw], F32, tag=f"gt_{self.name}")
            if t > 0:
                nc.vector.tensor_add(gates[:, :], ps[:, :], xg_t)
            else:
                nc.vector.tensor_copy(gates[:, :], xg_t)
            gsrc = gates

        acts = gate_p.tile([128, 3 * gw], F32, tag=f"ac_{self.name}")
        nc.scalar.activation(acts[:, :], gsrc[:, 0:3 * gw], AF.Sigmoid)
        nc.scalar.activation(self.state[:, 0:gw], gsrc[:, 3 * gw:4 * gw], AF.Tanh)
        prods = gate_p.tile([128, 2 * gw], F32, tag=f"pr_{self.name}")
        nc.vector.tensor_mul(prods[:, :], acts[:, 0:2 * gw], self.state[:, :])
        nc.vector.tensor_add(self.state[:, gw:2 * gw],
                             prods[:, 0:gw], prods[:, gw:2 * gw])
        tnc = gate_p.tile([128, gw], F32, tag=f"tn_{self.name}")
        nc.scalar.activation(tnc[:, :], self.state[:, gw:2 * gw], AF.Tanh)
        dst = self.ob[:, :, :, col_of_t(t)]
        nc.vector.tensor_mul(
            dst.rearrange("p kc b -> p (kc b)"), acts[:, 2 * gw:3 * gw], tnc[:, :])


def _body(tc, nc, d, out, xg_f, xg_b, xg_d, S, T, n, NS, NT, SN, TN, stages=5):
    n_sn = NS // SN
    n_tn = NT // TN

    with tc.tile_pool(name="root", bufs=1) as rp:
        ident = _load(nc, rp, d, "ident", (128, 128), BF16)

        # persistent across encoder->attention->decoder handoffs
        with tc.tile_pool(name="persist", bufs=1) as pp:
            melT = _load(nc, pp, d, "melT", (81, NT), BF16)
            te = pp.tile([128, 4 * NS], BF16, tag="te")
            ctx = pp.tile([128, 4 * NT], BF16, tag="ctx")

            # ---------------- phase E0: encoder gate precompute ------------
            with tc.tile_pool(name="e0w", bufs=1) as wp, \
                 tc.tile_pool(name="e0ps", bufs=4, space="PSUM") as psp, \
                 tc.tile_pool(name="e0sb", bufs=4) as sbp:
                tab_f = _load(nc, wp, d, "tab_f", (128, 4096), BF16)
                tab_b = _load(nc, wp, d, "tab_b", (128, 4096), BF16)
                oh_f = _load(nc, wp, d, "oh_f", (128, 2 * NS), BF16)
                oh_b = _load(nc, wp, d, "oh_b", (128, 2 * NS), BF16)
                _xg_produce(tc, nc, psp, sbp, tab_f, oh_f, xg_f, S, n, NS)
                _xg_produce(tc, nc, psp, sbp, tab_b, oh_b, xg_b, S, n, NS)
            if stages <= 1:
                return

            # ---------------- phase E1+E2: encoder recurrences + tproj -----
            with tc.tile_pool(name="e1buf", bufs=1) as ebp:
                buf_f = ebp.tile([128, 4 * NS], BF16, tag="buf_f")
                buf_b = ebp.tile([128, 4 * NS], BF16, tag="buf_b")

                with tc.tile_pool(name="e1w", bufs=1) as ewp, \
                     tc.tile_pool(name="e1ps", bufs=3, space="PSUM") as pls, \
                     tc.tile_pool(name="e1st", bufs=1) as stp, \
                     tc.tile_pool(name="e1gt", bufs=3) as gtp, \
                     tc.tile_pool(name="e1xg", bufs=2) as xgp:

                    whh_f = _load(nc, ewp, d, "whh_f", (128, 8192), BF16)
                    whh_b = _load(nc, ewp, d, "whh_b", (128, 8192), BF16)

                    CH = 128
                    cf = _Chain("f", whh_f, xg_f, buf_f, n, S, ident, stp, xgp, CH)
                    cb = _Chain("b", whh_b, xg_b, buf_b, n, S, ident, stp, xgp, CH)
                    cf.init(nc)
                    cb.init(nc)
                    for t in range(S):
                        cf.step(nc, t, pls, gtp, lambda u: u)
                        cb.step(nc, t, pls, gtp, lambda u: S - 1 - u)

                with tc.tile_pool(name="e2w", bufs=1) as wtp, \
                     tc.tile_pool(name="e2ps", bufs=4, space="PSUM") as ptp:
                    wt_f = _load(nc, wtp, d, "wt_f", (128, 2048), BF16)
                    wt_b = _load(nc, wtp, d, "wt_b", (128, 2048), BF16)
                    tb = _load(nc, wtp, d, "tb", (128, 4), F32)
                    for mc in range(4):
                        for nt in range(n_sn):
                            ps = ptp.tile([128, SN], F32, tag="ps")
                            for kc in range(4):
                                nc.tensor.matmul(
                                    ps[:, :],
                                    wt_f[:, (kc * 4 + mc) * 128:(kc * 4 + mc + 1) * 128],
                                    buf_f[:, kc * NS + nt * SN:kc * NS + (nt + 1) * SN],
                                    start=(kc == 0), stop=False)
                            for kc in range(4):
                                nc.tensor.matmul(
                                    ps[:, :],
                                    wt_b[:, (kc * 4 + mc) * 128:(kc * 4 + mc + 1) * 128],
                                    buf_b[:, kc * NS + nt * SN:kc * NS + (nt + 1) * SN],
                                    start=False, stop=(kc == 3))
                            _bias_copy(nc, mc * n_sn + nt,
                                       te[:, mc * NS + nt * SN:mc * NS + (nt + 1) * SN],
                                       ps[:, :], tb[:, mc:mc + 1])

            if stages <= 2:
                return
            # ---------------- phase A: attention ---------------------------
            _attention(tc, nc, d, te, ctx, melT, S, T, n, NS, NT, SN, TN)

            if stages <= 3:
                return
            # ---------------- phase X: decoder gate precompute --------------
            with tc.tile_pool(name="xw", bufs=1) as dwp, \
                 tc.tile_pool(name="xps", bufs=4, space="PSUM") as pdx, \
                 tc.tile_pool(name="xsb", bufs=4) as dsb:
                m1 = _load(nc, dwp, d, "m1", (81, 2048), BF16)
                m2 = _load(nc, dwp, d, "m2", (128, 8192), BF16)
                xgdv = xg_d.ap().rearrange("p (gc b t) -> p gc b t", gc=16, b=n)
                i = 0
                for gt in range(16):
                    for b in range(n):
                        for hf in range(T // TN):
                            span0 = b * T + hf * TN
                            ps = pdx.tile([128, TN], F32, tag="ps")
                            nc.tensor.matmul(
                                ps[:, :], m1[:, gt * 128:(gt + 1) * 128],
                                melT[:, span0:span0 + TN],
                                start=True, stop=False)
                            for kc in range(4):
                                nc.tensor.matmul(
                                    ps[:, :],
                                    m2[:, (kc * 16 + gt) * 128:(kc * 16 + gt + 1) * 128],
                                    ctx[:, kc * NT + span0:kc * NT + span0 + TN],
                                    start=False, stop=(kc == 3))
                            sb = dsb.tile([128, TN], BF16, tag="dx_sb")
                            _plain_copy(nc, i, sb[:, :], ps[:, :])
                            i += 1
                            nc.sync.dma_start(
                                xgdv[:, gt, b, hf * TN:(hf + 1) * TN], sb[:, :])

            if stages <= 4:
                return
        # persist pool (melT/te/ctx) closes here; decoder only needs xg_d
        # ---------------- phase D: decoder recurrence + out proj ----------
        with tc.tile_pool(name="dbufp", bufs=1) as dbp:
            dbuf = dbp.tile([128, 4 * NT], BF16, tag="dbuf")
            with tc.tile_pool(name="dw", bufs=1) as dwp, \
                 tc.tile_pool(name="dps", bufs=6, space="PSUM") as pls, \
                 tc.tile_pool(name="dst", bufs=1) as stp, \
                 tc.tile_pool(name="dgt", bufs=3) as gtp, \
                 tc.tile_pool(name="dxg", bufs=2) as xgp:

                whh_d = _load(nc, dwp, d, "whh_d", (128, 8192), BF16)
                cd = _Chain("d", whh_d, xg_d, dbuf, n, T, ident, stp, xgp, 250)
                cd.init(nc)
                for t in range(T):
                    cd.step(nc, t, pls, gtp, lambda u: u)

            with tc.tile_pool(name="ow", bufs=1) as mop, \
                 tc.tile_pool(name="ops", bufs=4, space="PSUM") as pso, \
                 tc.tile_pool(name="osb", bufs=4) as sbo:
                mo = _load(nc, mop, d, "mo", (128, 320), BF16)
                mob = _load(nc, mop, d, "mob", (80, 1), F32)
                for nt in range(n_tn):
                    ps = pso.tile([80, TN], F32, tag="ps")
                    for kc in range(4):
                        nc.tensor.matmul(
                            ps[:, :], mo[:, kc * 80:(kc + 1) * 80],
                            dbuf[:, kc * NT + nt * TN:kc * NT + (nt + 1) * TN],
                            start=(kc == 0), stop=(kc == 3))
                    sb = sbo.tile([80, TN], F32, tag="out_sb")
                    nc.scalar.activation(sb[:, :], ps[:, :], AF.Identity,
                                         bias=mob[:, :])
                    nc.sync.dma_start(out.ap()[:, nt * TN:(nt + 1) * TN], sb[:, :])


def _attention(tc, nc, d, te, ctx, melT, S, T, n, NS, NT, SN, TN):
    n_tn = NT // TN
    n_sc = NS // 128
    with tc.tile_pool(name="aw", bufs=1) as awp, \
         tc.tile_pool(name="aps", bufs=4, space="PSUM") as pa, \
         tc.tile_pool(name="aqt", bufs=1) as qtp, \
         tc.tile_pool(name="akt", bufs=1) as ktp, \
         tc.tile_pool(name="avs", bufs=1) as vsp, \
         tc.tile_pool(name="aet", bufs=2) as etp, \
         tc.tile_pool(name="actx", bufs=3) as cxp:

        wk = _load(nc, awp, d, "wk", (128, 2048), BF16)
        wv = _load(nc, awp, d, "wv", (128, 2048), BF16)
        vb = _load(nc, awp, d, "vb", (64, 8), F32)
        qw = _load(nc, awp, d, "qw", (81, 512), BF16)
        ones = awp.tile([1, 64], F32, tag="ones64")
        nc.vector.memset(ones[:, :], 1.0)

        # QT sbuf-resident: [128, 4mc x NT]
        qt = qtp.tile([128, 4 * NT], BF16, tag="qt")
        for mc in range(4):
            for nt in range(n_tn):
                ps = pa.tile([128, TN], F32, tag="ps")
                nc.tensor.matmul(ps[:, :], qw[:, mc * 128:(mc + 1) * 128],
                                 melT[:, nt * TN:(nt + 1) * TN],
                                 start=True, stop=True)
                _plain_copy(nc, mc * n_tn + nt,
                            qt[:, mc * NT + nt * TN:mc * NT + (nt + 1) * TN],
                            ps[:, :])

        # KT sbuf-resident: [128, 4mc x NS]
        kt = ktp.tile([128, 4 * NS], BF16, tag="kt")
        for mc in range(4):
            for nt in range(NS // SN):
                ps = pa.tile([128, SN], F32, tag="ps")
                for kc in range(4):
                    nc.tensor.matmul(
                        ps[:, :], wk[:, (kc * 4 + mc) * 128:(kc * 4 + mc + 1) * 128],
                        te[:, kc * NS + nt * SN:kc * NS + (nt + 1) * SN],
                        start=(kc == 0), stop=(kc == 3))
                _plain_copy(nc, mc * (NS // SN) + nt,
                            kt[:, mc * NS + nt * SN:mc * NS + (nt + 1) * SN],
                            ps[:, :])

        # V with ones column per head: [128(s-sub), n_sc x (8h x 65)]
        vsb = vsp.tile([128, n_sc * 520], BF16, tag="vsb")
        for sc in range(n_sc):
            ps = pa.tile([128, 512], F32, tag="ps")
            for kc in range(4):
                nc.tensor.matmul(
                    ps[:, :], te[:, kc * NS + sc * 128:kc * NS + sc * 128 + 128],
                    wv[:, kc * 512:(kc + 1) * 512],
                    start=(kc == 0), stop=(kc == 3))
            dst = vsb[:, sc * 520:(sc + 1) * 520].rearrange("p (h c) -> p h c", h=8)
            _plain_copy(nc, sc, dst[:, :, 0:64],
                        ps[:, :].rearrange("p (h c) -> p h c", h=8))
            nc.vector.memset(dst[:, :, 64:65], 1.0)

        # per (example, head): scoresT -> exp -> ctx + sums -> scale
        nsc_b = S // 128  # s-chunks per example
        for b in range(n):
            for h in range(NH):
                hc, hr = h // 2, (h % 2) * 64
                et = etp.tile([128, nsc_b * T], BF16, tag="et")
                for tt in range(T // TN):
                    qs = qt[hr:hr + 64,
                            hc * NT + b * T + tt * TN:hc * NT + b * T + (tt + 1) * TN]
                    for scl in range(nsc_b):
                        ps = pa.tile([128, TN], F32, tag="ps")
                        nc.tensor.matmul(
                            ps[:, :],
                            kt[hr:hr + 64,
                               hc * NS + b * S + scl * 128:hc * NS + b * S + scl * 128 + 128],
                            qs, start=True, stop=True)
                        nc.scalar.activation(
                            et[:, scl * T + tt * TN:scl * T + (tt + 1) * TN],
                            ps[:, :], AF.Exp)
                cps = [pa.tile([65, TN], F32, tag="ps", name=f"cps{tt}")
                       for tt in range(T // TN)]
                for scl in range(nsc_b):
                    lhs = vsb[:, (b * nsc_b + scl) * 520 + h * 65:
                              (b * nsc_b + scl) * 520 + (h + 1) * 65]
                    for tt in range(T // TN):
                        nc.tensor.matmul(cps[tt][:, :], lhs,
                                         et[:, scl * T + tt * TN:scl * T + (tt + 1) * TN],
                                         start=(scl == 0), stop=(scl == nsc_b - 1))
                for tt in range(T // TN):
                    rc = cxp.tile([1, TN], F32, tag="recip")
                    nc.vector.reciprocal(rc[:, :], cps[tt][64:65, :])
                    rb = pa.tile([64, TN], F32, tag="ps")
                    nc.tensor.matmul(rb[:, :], ones[:, :], rc[:, :],
                                     start=True, stop=True)
                    c0 = cxp.tile([64, TN], F32, tag="ctx_unsc")
                    _plain_copy(nc, b * NH + h + tt, c0[:, :], cps[tt][0:64, :])
                    sc1 = cxp.tile([64, TN], F32, tag="ctx_scaled")
                    nc.vector.tensor_mul(sc1[:, :], c0[:, :], rb[:, :])
                    nc.vector.tensor_scalar_add(
                        ctx[hr:hr + 64,
                            hc * NT + b * T + tt * TN:hc * NT + b * T + (tt + 1) * TN],
                        sc1[:, :], vb[:, h:h + 1])


# ---------------------------------------------------------------------------
# host wrapper
# ---------------------------------------------------------------------------

def prep_host(inputs, S, T, n_per_core, ncores):
    f32 = np.float32
    emb = np.asarray(inputs["emb"], f32)
    idx = np.asarray(inputs["phoneme_indices"]).astype(np.int64)
    mel = np.asarray(inputs["mel_specs"], f32)

    bias_f = np.asarray(inputs["enc_bih_f"], f32) + np.asarray(inputs["enc_bhh_f"], f32)
    bias_b = np.asarray(inputs["enc_bih_b"], f32) + np.asarray(inputs["enc_bhh_b"], f32)
    tab_f = _gate_cols(emb @ np.asarray(inputs["enc_Wih_f"], f32).T + bias_f)
    tab_b = _gate_cols(emb @ np.asarray(inputs["enc_Wih_b"], f32).T + bias_b)

    tproj_W = np.asarray(inputs["tproj_W"], f32)
    tproj_b = np.asarray(inputs["tproj_b"], f32)
    Wq, Wk, Wv = np.split(np.asarray(inputs["attn_in_W"], f32), 3, axis=0)
    bq, bk, bv = np.split(np.asarray(inputs["attn_in_b"], f32), 3)
    mpw = np.asarray(inputs["mproj_in_W"], f32)
    mpb = np.asarray(inputs["mproj_in_b"], f32)
    aow = np.asarray(inputs["attn_out_W"], f32)
    aob = np.asarray(inputs["attn_out_b"], f32)
    dWih = np.asarray(inputs["dec_Wih"], f32)
    dbias = np.asarray(inputs["dec_bih"], f32) + np.asarray(inputs["dec_bhh"], f32)
    mow = np.asarray(inputs["mproj_out_W"], f32)
    mob = np.asarray(inputs["mproj_out_b"], f32)

    scale = f32(1.0) / np.sqrt(f32(HD))
    QW = (Wq @ mpw) * scale
    qb_ = (bq + Wq @ mpb) * scale
    vb_ = bv                       # tproj_b folded into te; bk softmax-invariant
    W1, W2 = dWih[:, :H], dWih[:, H:]
    M1 = W1 @ mpw
    M2 = W2 @ aow
    dbias_ = dbias + W1 @ mpb + W2 @ aob

    def bf(a):
        return np.ascontiguousarray(a.astype(BF))

    def tab_tiles(tab):
        # [256, 2048] -> [128, (vc,gt)x128]
        t4 = tab.reshape(2, 128, 16, 128).transpose(1, 0, 2, 3)
        return t4.reshape(128, 2 * 16 * 128)

    m1h = np.concatenate([_gate_cols(M1.T), _gate_cols(dbias_).reshape(1, 2048)],
                         axis=0)

    common = {
        "tab_f": bf(tab_tiles(tab_f)),
        "tab_b": bf(tab_tiles(tab_b)),
        "whh_f": bf(_lhsT_tiles(_gate_cols(np.asarray(inputs["enc_Whh_f"], f32).T))),
        "whh_b": bf(_lhsT_tiles(_gate_cols(np.asarray(inputs["enc_Whh_b"], f32).T))),
        "whh_d": bf(_lhsT_tiles(_gate_cols(np.asarray(inputs["dec_Whh"], f32).T))),
        "wt_f": bf(_lhsT_tiles(tproj_W[:, :H].T)),
        "wt_b": bf(_lhsT_tiles(tproj_W[:, H:].T)),
        "tb": _bias_tiles(tproj_b),
        "wk": bf(_lhsT_tiles(Wk.T)),
        "wv": bf(Wv.T.reshape(4, 128, 512).transpose(1, 0, 2).reshape(128, 2048)),
        "vb": np.ascontiguousarray(vb_.reshape(8, 64).T.astype(f32)),
        "qw": bf(np.concatenate([QW.T, qb_.reshape(1, 512)], axis=0)),
        "m1": bf(m1h),
        "m2": bf(_lhsT_tiles(_gate_cols(M2.T))),
        "mo": bf(mow.T.reshape(4, 128, 80).transpose(1, 0, 2).reshape(128, 320)),
        "mob": np.ascontiguousarray(mob.reshape(80, 1).astype(f32)),
        "ident": bf(np.eye(128, dtype=f32)),
    }

    shifted = np.concatenate([np.zeros_like(mel[:, :1]), mel[:, :-1]], axis=1)

    in_maps = []
    for c in range(ncores):
        exs = list(range(c * n_per_core, (c + 1) * n_per_core))
        ohf = np.zeros((VOCAB, n_per_core * S), f32)
        ohb = np.zeros((VOCAB, n_per_core * S), f32)
        cols = np.arange(S)
        for bi, e in enumerate(exs):
            ohf[idx[e, :S], bi * S + cols] = 1.0
            ohb[idx[e, S - 1 - cols], bi * S + cols] = 1.0
        melTc = np.ones((MEL + 1, n_per_core * T), f32)
        for bi, e in enumerate(exs):
            melTc[:MEL, bi * T:(bi + 1) * T] = shifted[e, :T].T
        m = dict(common)
        m["oh_f"] = bf(np.concatenate([ohf[:128], ohf[128:]], axis=1))
        m["oh_b"] = bf(np.concatenate([ohb[:128], ohb[128:]], axis=1))
        m["melT"] = bf(melTc)
        in_maps.append(m)
    return in_maps


def run(inputs, S, T, n, ncores, trace=False):
    nc = build_program(S=S, T=T, n=n)
    in_maps = prep_host(inputs, S, T, n, ncores)
    res = bass_utils.run_bass_kernel_spmd(
        nc, in_maps, core_ids=list(range(ncores)), trace=trace)
    Bt = n * ncores
    out = np.zeros((Bt, T, MEL), np.float32)
    for c in range(ncores):
        o = np.asarray(res.results[c]["out"])
        for bi in range(n):
            out[c * n + bi] = o[:, bi * T:(bi + 1) * T].T
    return out, res


def kernel(**inputs):
    out, _ = run(inputs, S_FULL, T_FULL, B_FULL // NCORES, NCORES)
    return out
